# revision 13
# baseline (speedup 1.0000x reference)
"""Self-contained Trainium2 Bass kernel for nn_Attention_395136991961.

Dense multi-head attention (B=8, N=1024, C=1024, H=16, D=64) with RoPE,
full materialized softmax, and output projection.

Sharding: data-parallel over batch B across the 8 NeuronCores (one batch
element per core, weights replicated, no collectives).

V3 design (vs the 467us fp32r baseline):
  - ALL matmuls in bf16: fp32r lowers to fp32_mode=HIGH (2-pass) on HW,
    measured 489ns vs 273ns per N=512 matmul. End-to-end bf16 error is
    ~5e-3, well under the 2e-2 gate.
  - QKV computed x-stationary with 512-wide moving W chunks (N=512 hides
    the ~151ns LDWEIGHTS; N=256 chains measured LDW-gated at 213ns/MM).
  - Q^T/K^T produced by DMA XBAR transpose (dma_start_transpose) of the
    RoPE'd token-major tiles: no PE transposes, no DVE drains.
  - Fused pipeline: QK chunks for pairs 4-7 issue between the attention
    blocks of pairs 0-3, so ACT (softmax exp, ~140us) overlaps the whole
    PE stream instead of a separate phase.
  - Softmax denominators ride as a ones-column in the V stationary
    (PV output row 64). Normalize: ACT copy (copy lives in the exp
    activation table - no table swap) -> DMA to partition 0 -> DVE
    reciprocal_approx_fast -> gpsimd partition_broadcast -> DVE mults.
    (reciprocal_approx_fast is a custom-DVE bitwise op: broken on HW for
    PSUM reads / partition-64 operands, hence the copy+DMA hop.)
  - PSUM: one shared work pool (QKV/proj [128,512] + S^T [128,2,512]
    allocs, bufs=2 -> 4 banks) + double-buffered PV accumulator
    (2x[128,2,512] -> 4 banks) = 8 banks exactly.
"""

import sys

if "/opt/trn_rl_repo" not in sys.path:
    sys.path.insert(0, "/opt/trn_rl_repo")

import numpy as np

import concourse.tile as tile
import concourse.mybir as mybir
from concourse import bacc
from concourse.bass_utils import run_bass_kernel_spmd

F32 = mybir.dt.float32
BF16 = mybir.dt.bfloat16
AF = mybir.ActivationFunctionType
OP = mybir.AluOpType

N_CORES = 8
C = 1024
H = 16
D = 64
HD2 = D // 2  # rotate-half split
SCALE = float(D) ** -0.5

PROFILE = False
LAST_EXEC_NS = None
_CACHE = {}


def build(n_tok):
    ntile = n_tok // 128          # token tiles (8)
    nct = C // 128                # contraction tiles (8)
    mch = 512                     # m-chunk for S columns
    npair = H // 2                # head pairs (8)

    nc = bacc.Bacc("TRN2", target_bir_lowering=False, debug=False, num_devices=1)

    xT = nc.dram_tensor("xT", [C, n_tok], BF16, kind="ExternalInput").ap()
    wT = nc.dram_tensor("wT", [C, 3 * C], BF16, kind="ExternalInput").ap()
    pwT = nc.dram_tensor("pwT", [C, C], BF16, kind="ExternalInput").ap()
    pbias = nc.dram_tensor("pbias", [1, C], F32, kind="ExternalInput").ap()
    cosN = nc.dram_tensor("cosN", [n_tok, D], F32, kind="ExternalInput").ap()
    sinA = nc.dram_tensor("sinA", [n_tok, D], F32, kind="ExternalInput").ap()
    vinit = nc.dram_tensor("vinit", [128, H * (D + 1)], BF16, kind="ExternalInput").ap()
    y = nc.dram_tensor("y", [n_tok, C], F32, kind="ExternalOutput").ap()

    xT_t = xT.rearrange("(t p) n -> p t n", p=128)
    wT_t = wT.rearrange("(t p) j -> p t j", p=128)
    pwT_t = pwT.rearrange("(t p) e -> p t e", p=128)
    cos_t = cosN.rearrange("(t p) d -> p t d", p=128)
    sin_t = sinA.rearrange("(t p) d -> p t d", p=128)

    with tile.TileContext(nc) as tc:
        with (
            tc.tile_pool(name="persist", bufs=1) as pp,
            tc.tile_pool(name="psA", bufs=2, space="PSUM") as psA,
            tc.tile_pool(name="psS", bufs=2, space="PSUM") as psS,
            tc.tile_pool(name="psPV", bufs=2, space="PSUM") as psPV,
            tc.tile_pool(name="wstream", bufs=2) as wsp,
            tc.tile_pool(name="ropetmp", bufs=2) as rtp,
            tc.tile_pool(name="qstage", bufs=3) as qsp,
            tc.tile_pool(name="ptpool", bufs=2) as ptp,
            tc.tile_pool(name="nrm", bufs=2) as nrm,
            tc.tile_pool(name="ypool", bufs=2) as yp,
        ):
            # ---------------- persistent tiles ----------------
            xT_sb = pp.tile([128, nct, n_tok], BF16, tag="xT")
            for ct in range(nct):
                nc.sync.dma_start(xT_sb[:, ct, :], xT_t[:, ct, :])
            qT_sb = pp.tile([128, npair, n_tok], BF16, tag="qT")
            kT_sb = pp.tile([128, npair, n_tok], BF16, tag="kT")
            v_sb = pp.tile([128, ntile, H, D + 1], BF16, tag="v")
            for t in range(ntile):
                nc.sync.dma_start(
                    v_sb[:, t, :, :].rearrange("p h d -> p (h d)"), vinit[:]
                )
            oT_sb = pp.tile([128, nct, n_tok], BF16, tag="oT")
            cos_sb = pp.tile([128, ntile, D], F32, tag="cos")
            nc.sync.dma_start(cos_sb[:], cos_t)
            sin_sb = pp.tile([128, ntile, D], F32, tag="sin")
            nc.sync.dma_start(sin_sb[:], sin_t)
            bias_b = pp.tile([128, C], F32, tag="biasb")
            pb_sb = pp.tile([1, C], F32, tag="pb")
            nc.sync.dma_start(pb_sb[:], pbias[:])
            nc.gpsimd.partition_broadcast(bias_b[:], pb_sb[0:1, :])
            pwc = pp.tile([128, nct, 2, 512], BF16, tag="pw")

            # -------------- 512-wide x-stationary chunk ---------------------
            def qkv_chunk(which, half):
                # which: 0=q 1=k 2=v; half: 0/1 -> features 512*half..+512
                # q/k chunks cover pairs 4*half..4*half+4 and get RoPE +
                # DMA-transpose into qT_sb/kT_sb; v chunks drain to v_sb.
                w = wsp.tile([128, nct, 512], BF16, tag="w")
                base = which * C + half * 512
                nc.sync.dma_start(w[:], wT_t[:, :, base : base + 512])
                for t in range(ntile):
                    pq = psA.tile([128, 512], F32, tag="pA")
                    for ct in range(nct):
                        nc.tensor.matmul(
                            pq[:],
                            xT_sb[:, ct, t * 128 : (t + 1) * 128],
                            w[:, ct, :],
                            start=(ct == 0),
                            stop=(ct == nct - 1),
                        )
                    if which == 2:
                        nc.vector.tensor_copy(
                            v_sb[:, t, half * 8 : half * 8 + 8, 0:D],
                            pq[:].rearrange("p (h d) -> p h d", d=D),
                        )
                        continue
                    pq3 = pq[:].rearrange("p (h d) -> p h d", d=D)
                    cos3 = (
                        cos_sb[:, t, :]
                        .rearrange("p (o d) -> p o d", d=D)
                        .to_broadcast([128, 8, D])
                    )
                    sinlo = (
                        sin_sb[:, t, 0:HD2]
                        .rearrange("p (o d) -> p o d", d=HD2)
                        .to_broadcast([128, 8, HD2])
                    )
                    sinhi = (
                        sin_sb[:, t, HD2:D]
                        .rearrange("p (o d) -> p o d", d=HD2)
                        .to_broadcast([128, 8, HD2])
                    )
                    tmp = rtp.tile([128, 512], F32, tag="rt")
                    tmp3 = tmp[:].rearrange("p (h d) -> p h d", d=D)
                    nc.vector.tensor_tensor(
                        out=tmp3[:, :, 0:HD2], in0=pq3[:, :, HD2:D],
                        in1=sinlo, op=OP.mult,
                    )
                    nc.vector.tensor_tensor(
                        out=tmp3[:, :, HD2:D], in0=pq3[:, :, 0:HD2],
                        in1=sinhi, op=OP.mult,
                    )
                    u = rtp.tile([128, 512], F32, tag="ru")
                    nc.vector.tensor_tensor(
                        out=u[:].rearrange("p (h d) -> p h d", d=D),
                        in0=pq3, in1=cos3, op=OP.mult,
                    )
                    qh = qsp.tile([128, 512], BF16, tag="qh")
                    nc.vector.tensor_tensor(
                        out=qh[:], in0=u[:], in1=tmp[:], op=OP.add
                    )
                    dstT = qT_sb if which == 0 else kT_sb
                    nc.sync.dma_start_transpose(
                        dstT[:, 4 * half : 4 * half + 4, t * 128 : (t + 1) * 128],
                        qh[:],
                    )

            # -------------- S^T + exp for one (pair, mc) --------------------
            def s_exp(p, mc):
                pT = ptp.tile([128, ntile, 2, 512], BF16, tag="pT")
                ms = mc * mch
                for t in range(ntile):
                    psE = psS.tile([128, 512], F32, tag="ps")
                    psO = psS.tile([128, 512], F32, tag="ps")
                    nc.tensor.matmul(
                        psE[:],
                        kT_sb[0:64, p, t * 128 : (t + 1) * 128],
                        qT_sb[0:64, p, ms : ms + mch],
                        start=True,
                        stop=True,
                    )
                    nc.tensor.matmul(
                        psO[:],
                        kT_sb[64:128, p, t * 128 : (t + 1) * 128],
                        qT_sb[64:128, p, ms : ms + mch],
                        start=True,
                        stop=True,
                    )
                    nc.scalar.activation(
                        pT[:, t, 0, :], psE[:], AF.Exp, scale=SCALE
                    )
                    nc.scalar.activation(
                        pT[:, t, 1, :], psO[:], AF.Exp, scale=SCALE
                    )
                return pT

            # -------------- PV + normalize for one (pair, mc) ---------------
            def pv_norm(p, mc, pT):
                ms = mc * mch
                po = psPV.tile([128, 2, 512], F32, tag="po")
                for t in range(ntile):
                    nc.tensor.matmul(
                        po[0:65, 0, :],
                        v_sb[:, t, 2 * p, :],
                        pT[:, t, 0, :],
                        start=(t == 0),
                        stop=(t == ntile - 1),
                    )
                    nc.tensor.matmul(
                        po[0:65, 1, :],
                        v_sb[:, t, 2 * p + 1, :],
                        pT[:, t, 1, :],
                        start=(t == 0),
                        stop=(t == ntile - 1),
                    )
                ssb = nrm.tile([65, 2, 512], F32, tag="ssb")
                nc.scalar.copy(ssb[64:65, :, :], po[64:65, :, :])
                s0 = nrm.tile([1, 2, 512], F32, tag="s0")
                nc.sync.dma_start(s0[:], ssb[64:65, :, :])
                rs0 = nrm.tile([1, 2, 512], F32, tag="rs0")
                nc.vector.reciprocal_approx_fast(out=rs0[:], in_=s0[:])
                rbi = nrm.tile([64, 2, 512], F32, tag="rbi")
                nc.gpsimd.partition_broadcast(rbi[:], rs0[0:1, :, :])
                nc.vector.tensor_tensor(
                    out=oT_sb[0:64, p, ms : ms + mch],
                    in0=po[0:64, 0, :],
                    in1=rbi[:, 0, :],
                    op=OP.mult,
                )
                tmpo = nrm.tile([64, 512], BF16, tag="tmpo")
                nc.vector.tensor_tensor(
                    out=tmpo[:],
                    in0=po[0:64, 1, :],
                    in1=rbi[:, 1, :],
                    op=OP.mult,
                )
                nc.sync.dma_start(oT_sb[64:128, p, ms : ms + mch], tmpo[:])

            # -------------- fused pipeline ----------------------------------
            qkv_chunk(0, 0)
            qkv_chunk(1, 0)
            qkv_chunk(2, 0)
            for p in range(npair):
                if p == 0:
                    qkv_chunk(0, 1)
                if p == 1:
                    qkv_chunk(1, 1)
                if p == 2:
                    qkv_chunk(2, 1)
                if p == 3:
                    nc.sync.dma_start(
                        pwc[:], pwT_t.rearrange("p t (a e) -> p t a e", a=2)
                    )
                pT0 = s_exp(p, 0)
                pT1 = s_exp(p, 1)
                pv_norm(p, 0, pT0)
                pv_norm(p, 1, pT1)

            # -------------- proj tail ---------------------------------------
            for t in range(ntile):
                for ec in range(2):
                    py = psA.tile([128, 512], F32, tag="pA")
                    for ft in range(nct):
                        nc.tensor.matmul(
                            py[:],
                            oT_sb[:, ft, t * 128 : (t + 1) * 128],
                            pwc[:, ft, ec, :],
                            start=(ft == 0),
                            stop=(ft == nct - 1),
                        )
                    ysb = yp.tile([128, 512], F32, tag="y")
                    nc.vector.tensor_tensor(
                        out=ysb[:],
                        in0=py[:],
                        in1=bias_b[:, ec * 512 : (ec + 1) * 512],
                        op=OP.add,
                    )
                    nc.sync.dma_start(
                        y[t * 128 : (t + 1) * 128, ec * 512 : (ec + 1) * 512],
                        ysb[:],
                    )

    nc.compile()
    return nc


def _host_inputs(x, rope_freqs, qkv_w, proj_w, proj_b):
    import ml_dtypes

    x = np.asarray(x, dtype=np.float32)
    rope_freqs = np.asarray(rope_freqs, dtype=np.float32)
    qkv_w = np.asarray(qkv_w, dtype=np.float32)
    proj_w = np.asarray(proj_w, dtype=np.float32)
    proj_b = np.asarray(proj_b, dtype=np.float32)

    B, n_tok, _ = x.shape
    wTh = np.ascontiguousarray(qkv_w.T).astype(ml_dtypes.bfloat16)
    pwTh = np.ascontiguousarray(proj_w.T).astype(ml_dtypes.bfloat16)
    freqs = rope_freqs[0, :, 0, :]  # [N, D]
    cosh = np.cos(freqs).astype(np.float32)
    sinh = np.sin(freqs).astype(np.float32)
    sinAh = np.concatenate([-sinh[:, :HD2], sinh[:, HD2:]], axis=1)
    sinAh = np.ascontiguousarray(sinAh)
    vinith = np.zeros((128, H, D + 1), np.float32)
    vinith[:, :, D] = 1.0
    vinith = vinith.reshape(128, H * (D + 1)).astype(ml_dtypes.bfloat16)
    pbh = np.ascontiguousarray(proj_b.reshape(1, C))

    in_maps = []
    for b in range(B):
        in_maps.append(
            {
                "xT": np.ascontiguousarray(x[b].T).astype(ml_dtypes.bfloat16),
                "wT": wTh,
                "pwT": pwTh,
                "pbias": pbh,
                "cosN": cosh,
                "sinA": sinAh,
                "vinit": vinith,
            }
        )
    return in_maps, n_tok


def kernel(x, rope_freqs, qkv_w, proj_w, proj_b):
    global LAST_EXEC_NS
    in_maps, n_tok = _host_inputs(x, rope_freqs, qkv_w, proj_w, proj_b)
    key = ("nc", n_tok)
    if key not in _CACHE:
        _CACHE[key] = build(n_tok)
    nc = _CACHE[key]

    trace = False
    if PROFILE:
        try:
            import profshim

            profshim.install()
            trace = True
        except Exception:
            trace = False

    res = run_bass_kernel_spmd(
        nc, in_maps, list(range(len(in_maps))), trace=trace
    )
    LAST_EXEC_NS = res.exec_time_ns
    out = np.stack([res.results[b]["y"] for b in range(len(in_maps))], axis=0)
    return out


# revision 16
# speedup vs baseline: 1.3315x; 1.3315x over previous
"""Self-contained Trainium2 Bass kernel for nn_Attention_395136991961.

Dense multi-head attention (B=8, N=1024, C=1024, H=16, D=64) with RoPE,
full materialized softmax, and output projection.

Sharding: data-parallel over batch B across the 8 NeuronCores (one batch
element per core, weights replicated, no collectives).

V3 design (vs the 467us fp32r baseline):
  - ALL matmuls in bf16: fp32r lowers to fp32_mode=HIGH (2-pass) on HW,
    measured 489ns vs 273ns per N=512 matmul. End-to-end bf16 error is
    ~5e-3, well under the 2e-2 gate.
  - QKV computed x-stationary with 512-wide moving W chunks (N=512 hides
    the ~151ns LDWEIGHTS; N=256 chains measured LDW-gated at 213ns/MM).
  - Q^T/K^T produced by DMA XBAR transpose (dma_start_transpose) of the
    RoPE'd token-major tiles: no PE transposes, no DVE drains.
  - Fused pipeline: QK chunks for pairs 4-7 issue between the attention
    blocks of pairs 0-3, so ACT (softmax exp, ~140us) overlaps the whole
    PE stream instead of a separate phase.
  - Softmax denominators ride as a ones-column in the V stationary
    (PV output row 64). Normalize: ACT copy (copy lives in the exp
    activation table - no table swap) -> DMA to partition 0 -> DVE
    reciprocal_approx_fast -> gpsimd partition_broadcast -> DVE mults.
    (reciprocal_approx_fast is a custom-DVE bitwise op: broken on HW for
    PSUM reads / partition-64 operands, hence the copy+DMA hop.)
  - PSUM: one shared work pool (QKV/proj [128,512] + S^T [128,2,512]
    allocs, bufs=2 -> 4 banks) + double-buffered PV accumulator
    (2x[128,2,512] -> 4 banks) = 8 banks exactly.
"""

import sys

if "/opt/trn_rl_repo" not in sys.path:
    sys.path.insert(0, "/opt/trn_rl_repo")

import numpy as np

import concourse.tile as tile
import concourse.mybir as mybir
from concourse import bacc
from concourse.bass_utils import run_bass_kernel_spmd

F32 = mybir.dt.float32
BF16 = mybir.dt.bfloat16
AF = mybir.ActivationFunctionType
OP = mybir.AluOpType

N_CORES = 8
C = 1024
H = 16
D = 64
HD2 = D // 2  # rotate-half split
SCALE = float(D) ** -0.5

PROFILE = False
LAST_EXEC_NS = None
_CACHE = {}


def build(n_tok):
    ntile = n_tok // 128          # token tiles (8)
    nct = C // 128                # contraction tiles (8)
    mch = 512                     # m-chunk for S columns
    npair = H // 2                # head pairs (8)

    nc = bacc.Bacc("TRN2", target_bir_lowering=False, debug=False, num_devices=1)

    xT = nc.dram_tensor("xT", [C, n_tok], BF16, kind="ExternalInput").ap()
    wT = nc.dram_tensor("wT", [C, 3 * C], BF16, kind="ExternalInput").ap()
    pwT = nc.dram_tensor("pwT", [C, C], BF16, kind="ExternalInput").ap()
    pbias = nc.dram_tensor("pbias", [1, C], F32, kind="ExternalInput").ap()
    cosN = nc.dram_tensor("cosN", [n_tok, D], F32, kind="ExternalInput").ap()
    sinA = nc.dram_tensor("sinA", [n_tok, D], F32, kind="ExternalInput").ap()
    vinit = nc.dram_tensor("vinit", [128, H * (D + 1)], BF16, kind="ExternalInput").ap()
    y = nc.dram_tensor("y", [n_tok, C], F32, kind="ExternalOutput").ap()

    xT_t = xT.rearrange("(t p) n -> p t n", p=128)
    wT_t = wT.rearrange("(t p) j -> p t j", p=128)
    pwT_t = pwT.rearrange("(t p) e -> p t e", p=128)
    cos_t = cosN.rearrange("(t p) d -> p t d", p=128)
    sin_t = sinA.rearrange("(t p) d -> p t d", p=128)

    with tile.TileContext(nc) as tc:
        with (
            tc.tile_pool(name="persist", bufs=1) as pp,
            tc.tile_pool(name="psA", bufs=2, space="PSUM") as psA,
            tc.tile_pool(name="psS", bufs=2, space="PSUM") as psS,
            tc.tile_pool(name="psPV", bufs=1, space="PSUM") as psPV,
            tc.tile_pool(name="wstream", bufs=2) as wsp,
            tc.tile_pool(name="ropetmp", bufs=2) as rtp,
            tc.tile_pool(name="qstage", bufs=3) as qsp,
            tc.tile_pool(name="ptpool", bufs=2) as ptp,
            tc.tile_pool(name="nrm", bufs=2) as nrm,
            tc.tile_pool(name="ypool", bufs=2) as yp,
        ):
            # ---------------- persistent tiles ----------------
            xT_sb = pp.tile([128, nct, n_tok], BF16, tag="xT")
            for ct in range(nct):
                nc.sync.dma_start(xT_sb[:, ct, :], xT_t[:, ct, :])
            qT_sb = pp.tile([128, npair, n_tok], BF16, tag="qT")
            kT_sb = pp.tile([128, npair, n_tok], BF16, tag="kT")
            v_sb = pp.tile([128, ntile, H, D + 1], BF16, tag="v")
            for t in range(ntile):
                nc.sync.dma_start(
                    v_sb[:, t, :, :].rearrange("p h d -> p (h d)"), vinit[:]
                )
            oT_sb = pp.tile([128, nct, n_tok], BF16, tag="oT")
            cos_sb = pp.tile([128, ntile, D], F32, tag="cos")
            nc.sync.dma_start(cos_sb[:], cos_t)
            sin_sb = pp.tile([128, ntile, D], F32, tag="sin")
            nc.sync.dma_start(sin_sb[:], sin_t)
            bias_b = pp.tile([128, C], F32, tag="biasb")
            pb_sb = pp.tile([1, C], F32, tag="pb")
            nc.sync.dma_start(pb_sb[:], pbias[:])
            nc.gpsimd.partition_broadcast(bias_b[:], pb_sb[0:1, :])
            pwc = pp.tile([128, nct, 2, 512], BF16, tag="pw")

            # -------------- 512-wide x-stationary chunk ---------------------
            def qkv_chunk(which, half):
                # which: 0=q 1=k 2=v; half: 0/1 -> features 512*half..+512
                # q/k chunks cover pairs 4*half..4*half+4 and get RoPE +
                # DMA-transpose into qT_sb/kT_sb; v chunks drain to v_sb.
                w = wsp.tile([128, nct, 512], BF16, tag="w")
                base = which * C + half * 512
                nc.sync.dma_start(w[:], wT_t[:, :, base : base + 512])
                for t in range(ntile):
                    pq = psA.tile([128, 512], F32, tag="pA")
                    for ct in range(nct):
                        nc.tensor.matmul(
                            pq[:],
                            xT_sb[:, ct, t * 128 : (t + 1) * 128],
                            w[:, ct, :],
                            start=(ct == 0),
                            stop=(ct == nct - 1),
                        )
                    if which == 2:
                        nc.vector.tensor_copy(
                            v_sb[:, t, half * 8 : half * 8 + 8, 0:D],
                            pq[:].rearrange("p (h d) -> p h d", d=D),
                        )
                        continue
                    pq3 = pq[:].rearrange("p (h d) -> p h d", d=D)
                    cos3 = (
                        cos_sb[:, t, :]
                        .rearrange("p (o d) -> p o d", d=D)
                        .to_broadcast([128, 8, D])
                    )
                    sinlo = (
                        sin_sb[:, t, 0:HD2]
                        .rearrange("p (o d) -> p o d", d=HD2)
                        .to_broadcast([128, 8, HD2])
                    )
                    sinhi = (
                        sin_sb[:, t, HD2:D]
                        .rearrange("p (o d) -> p o d", d=HD2)
                        .to_broadcast([128, 8, HD2])
                    )
                    tmp = rtp.tile([128, 512], F32, tag="rt")
                    tmp3 = tmp[:].rearrange("p (h d) -> p h d", d=D)
                    nc.vector.tensor_tensor(
                        out=tmp3[:, :, 0:HD2], in0=pq3[:, :, HD2:D],
                        in1=sinlo, op=OP.mult,
                    )
                    nc.vector.tensor_tensor(
                        out=tmp3[:, :, HD2:D], in0=pq3[:, :, 0:HD2],
                        in1=sinhi, op=OP.mult,
                    )
                    u = rtp.tile([128, 512], F32, tag="ru")
                    nc.vector.tensor_tensor(
                        out=u[:].rearrange("p (h d) -> p h d", d=D),
                        in0=pq3, in1=cos3, op=OP.mult,
                    )
                    qh = qsp.tile([128, 512], BF16, tag="qh")
                    nc.vector.tensor_tensor(
                        out=qh[:], in0=u[:], in1=tmp[:], op=OP.add
                    )
                    dstT = qT_sb if which == 0 else kT_sb
                    nc.sync.dma_start_transpose(
                        dstT[:, 4 * half : 4 * half + 4, t * 128 : (t + 1) * 128],
                        qh[:],
                    )

            # -------------- S^T + exp for one (pair, mc) --------------------
            def s_exp(p, mc):
                pT = ptp.tile([128, ntile, 2, 512], BF16, tag="pT")
                ms = mc * mch
                for t in range(ntile):
                    ps = psS.tile([128, 2, 512], F32, tag="ps")
                    nc.tensor.matmul(
                        ps[:, 0, :],
                        kT_sb[0:64, p, t * 128 : (t + 1) * 128],
                        qT_sb[0:64, p, ms : ms + mch],
                        start=True,
                        stop=True,
                    )
                    nc.tensor.matmul(
                        ps[:, 1, :],
                        kT_sb[64:128, p, t * 128 : (t + 1) * 128],
                        qT_sb[64:128, p, ms : ms + mch],
                        start=True,
                        stop=True,
                    )
                    nc.scalar.activation(
                        pT[:, t, :, :].rearrange("p a m -> p (a m)"),
                        ps[:].rearrange("p a m -> p (a m)"),
                        AF.Exp,
                        scale=SCALE,
                    )
                return pT

            # -------------- PV + normalize for one (pair, mc) ---------------
            def pv_norm(p, mc, pT):
                ms = mc * mch
                po = psPV.tile([128, 2, 512], F32, tag="po")
                for t in range(ntile):
                    nc.tensor.matmul(
                        po[0:65, 0, :],
                        v_sb[:, t, 2 * p, :],
                        pT[:, t, 0, :],
                        start=(t == 0),
                        stop=(t == ntile - 1),
                    )
                    nc.tensor.matmul(
                        po[0:65, 1, :],
                        v_sb[:, t, 2 * p + 1, :],
                        pT[:, t, 1, :],
                        start=(t == 0),
                        stop=(t == ntile - 1),
                    )
                with tc.high_priority():
                    ssb = nrm.tile([65, 2, 512], F32, tag="ssb")
                    nc.scalar.copy(ssb[64:65, :, :], po[64:65, :, :])
                    s0 = nrm.tile([1, 2, 512], F32, tag="s0")
                    nc.sync.dma_start(s0[:], ssb[64:65, :, :])
                    rs0 = nrm.tile([1, 2, 512], F32, tag="rs0")
                    nc.vector.reciprocal_approx_fast(out=rs0[:], in_=s0[:])
                    rbi = nrm.tile([64, 2, 512], F32, tag="rbi")
                    nc.gpsimd.partition_broadcast(rbi[:], rs0[0:1, :, :])
                    nc.vector.tensor_tensor(
                        out=oT_sb[0:64, p, ms : ms + mch],
                        in0=po[0:64, 0, :],
                        in1=rbi[:, 0, :],
                        op=OP.mult,
                    )
                    tmpo = nrm.tile([64, 512], BF16, tag="tmpo")
                    nc.vector.tensor_tensor(
                        out=tmpo[:],
                        in0=po[0:64, 1, :],
                        in1=rbi[:, 1, :],
                        op=OP.mult,
                    )
                    nc.sync.dma_start(oT_sb[64:128, p, ms : ms + mch], tmpo[:])

            # -------------- fused pipeline ----------------------------------
            qkv_chunk(0, 0)
            qkv_chunk(1, 0)
            qkv_chunk(2, 0)
            for p in range(npair):
                if p == 0:
                    qkv_chunk(0, 1)
                if p == 1:
                    qkv_chunk(1, 1)
                if p == 2:
                    qkv_chunk(2, 1)
                if p == 3:
                    nc.sync.dma_start(
                        pwc[:], pwT_t.rearrange("p t (a e) -> p t a e", a=2)
                    )
                pT0 = s_exp(p, 0)
                pT1 = s_exp(p, 1)
                pv_norm(p, 0, pT0)
                pv_norm(p, 1, pT1)

            # -------------- proj tail ---------------------------------------
            for t in range(ntile):
                for ec in range(2):
                    py = psA.tile([128, 512], F32, tag="pA")
                    for ft in range(nct):
                        nc.tensor.matmul(
                            py[:],
                            oT_sb[:, ft, t * 128 : (t + 1) * 128],
                            pwc[:, ft, ec, :],
                            start=(ft == 0),
                            stop=(ft == nct - 1),
                        )
                    ysb = yp.tile([128, 512], F32, tag="y")
                    nc.vector.tensor_tensor(
                        out=ysb[:],
                        in0=py[:],
                        in1=bias_b[:, ec * 512 : (ec + 1) * 512],
                        op=OP.add,
                    )
                    nc.sync.dma_start(
                        y[t * 128 : (t + 1) * 128, ec * 512 : (ec + 1) * 512],
                        ysb[:],
                    )

    nc.compile()
    return nc


def _host_inputs(x, rope_freqs, qkv_w, proj_w, proj_b):
    import ml_dtypes

    x = np.asarray(x, dtype=np.float32)
    rope_freqs = np.asarray(rope_freqs, dtype=np.float32)
    qkv_w = np.asarray(qkv_w, dtype=np.float32)
    proj_w = np.asarray(proj_w, dtype=np.float32)
    proj_b = np.asarray(proj_b, dtype=np.float32)

    B, n_tok, _ = x.shape
    wTh = np.ascontiguousarray(qkv_w.T).astype(ml_dtypes.bfloat16)
    pwTh = np.ascontiguousarray(proj_w.T).astype(ml_dtypes.bfloat16)
    freqs = rope_freqs[0, :, 0, :]  # [N, D]
    cosh = np.cos(freqs).astype(np.float32)
    sinh = np.sin(freqs).astype(np.float32)
    sinAh = np.concatenate([-sinh[:, :HD2], sinh[:, HD2:]], axis=1)
    sinAh = np.ascontiguousarray(sinAh)
    vinith = np.zeros((128, H, D + 1), np.float32)
    vinith[:, :, D] = 1.0
    vinith = vinith.reshape(128, H * (D + 1)).astype(ml_dtypes.bfloat16)
    pbh = np.ascontiguousarray(proj_b.reshape(1, C))

    in_maps = []
    for b in range(B):
        in_maps.append(
            {
                "xT": np.ascontiguousarray(x[b].T).astype(ml_dtypes.bfloat16),
                "wT": wTh,
                "pwT": pwTh,
                "pbias": pbh,
                "cosN": cosh,
                "sinA": sinAh,
                "vinit": vinith,
            }
        )
    return in_maps, n_tok


def kernel(x, rope_freqs, qkv_w, proj_w, proj_b):
    global LAST_EXEC_NS
    in_maps, n_tok = _host_inputs(x, rope_freqs, qkv_w, proj_w, proj_b)
    key = ("nc", n_tok)
    if key not in _CACHE:
        _CACHE[key] = build(n_tok)
    nc = _CACHE[key]

    trace = False
    if PROFILE:
        try:
            import profshim

            profshim.install()
            trace = True
        except Exception:
            trace = False

    res = run_bass_kernel_spmd(
        nc, in_maps, list(range(len(in_maps))), trace=trace
    )
    LAST_EXEC_NS = res.exec_time_ns
    out = np.stack([res.results[b]["y"] for b in range(len(in_maps))], axis=0)
    return out


# revision 21
# speedup vs baseline: 1.4634x; 1.0990x over previous
"""Self-contained Trainium2 Bass kernel for nn_Attention_395136991961.

Dense multi-head attention (B=8, N=1024, C=1024, H=16, D=64) with RoPE,
full materialized softmax, and output projection.

Sharding: data-parallel over batch B across the 8 NeuronCores (one batch
element per core, weights replicated, no collectives).

V3 design (vs the 467us fp32r baseline):
  - ALL matmuls in bf16: fp32r lowers to fp32_mode=HIGH (2-pass) on HW,
    measured 489ns vs 273ns per N=512 matmul. End-to-end bf16 error is
    ~5e-3, well under the 2e-2 gate.
  - QKV computed x-stationary with 512-wide moving W chunks (N=512 hides
    the ~151ns LDWEIGHTS; N=256 chains measured LDW-gated at 213ns/MM).
  - Q^T/K^T produced by DMA XBAR transpose (dma_start_transpose) of the
    RoPE'd token-major tiles: no PE transposes, no DVE drains.
  - Fused pipeline: QK chunks for pairs 4-7 issue between the attention
    blocks of pairs 0-3, so ACT (softmax exp, ~140us) overlaps the whole
    PE stream instead of a separate phase.
  - Softmax denominators ride as a ones-column in the V stationary
    (PV output row 64). Normalize: ACT copy (copy lives in the exp
    activation table - no table swap) -> DMA to partition 0 -> DVE
    reciprocal_approx_fast -> gpsimd partition_broadcast -> DVE mults.
    (reciprocal_approx_fast is a custom-DVE bitwise op: broken on HW for
    PSUM reads / partition-64 operands, hence the copy+DMA hop.)
  - PSUM: one shared work pool (QKV/proj [128,512] + S^T [128,2,512]
    allocs, bufs=2 -> 4 banks) + double-buffered PV accumulator
    (2x[128,2,512] -> 4 banks) = 8 banks exactly.
"""

import sys

if "/opt/trn_rl_repo" not in sys.path:
    sys.path.insert(0, "/opt/trn_rl_repo")

import numpy as np

import concourse.tile as tile
import concourse.mybir as mybir
from concourse import bacc
from concourse.bass_utils import run_bass_kernel_spmd

F32 = mybir.dt.float32
BF16 = mybir.dt.bfloat16
AF = mybir.ActivationFunctionType
OP = mybir.AluOpType

N_CORES = 8
C = 1024
H = 16
D = 64
HD2 = D // 2  # rotate-half split
SCALE = float(D) ** -0.5

PROFILE = False
LAST_EXEC_NS = None
_CACHE = {}


def build(n_tok):
    ntile = n_tok // 128          # token tiles (8)
    nct = C // 128                # contraction tiles (8)
    mch = 512                     # m-chunk for S columns
    npair = H // 2                # head pairs (8)

    nc = bacc.Bacc("TRN2", target_bir_lowering=False, debug=False, num_devices=1)

    xT = nc.dram_tensor("xT", [C, n_tok], BF16, kind="ExternalInput").ap()
    wT = nc.dram_tensor("wT", [C, 3 * C], BF16, kind="ExternalInput").ap()
    pwT = nc.dram_tensor("pwT", [C, C], BF16, kind="ExternalInput").ap()
    pbias = nc.dram_tensor("pbias", [1, C], F32, kind="ExternalInput").ap()
    cosN = nc.dram_tensor("cosN", [n_tok, D], F32, kind="ExternalInput").ap()
    sinA = nc.dram_tensor("sinA", [n_tok, D], F32, kind="ExternalInput").ap()
    y = nc.dram_tensor("y", [n_tok, C], F32, kind="ExternalOutput").ap()

    xT_t = xT.rearrange("(t p) n -> p t n", p=128)
    wT_t = wT.rearrange("(t p) j -> p t j", p=128)
    pwT_t = pwT.rearrange("(t p) e -> p t e", p=128)
    cos_t = cosN.rearrange("(t p) d -> p t d", p=128)
    sin_t = sinA.rearrange("(t p) d -> p t d", p=128)

    with tile.TileContext(nc) as tc:
        with (
            tc.tile_pool(name="persist", bufs=1) as pp,
            tc.tile_pool(name="psA", bufs=2, space="PSUM") as psA,
            tc.tile_pool(name="psS", bufs=2, space="PSUM") as psS,
            tc.tile_pool(name="psPV", bufs=1, space="PSUM") as psPV,
            tc.tile_pool(name="wstream", bufs=2) as wsp,
            tc.tile_pool(name="ropetmp", bufs=2) as rtp,
            tc.tile_pool(name="qstage", bufs=3) as qsp,
            tc.tile_pool(name="ptpool", bufs=2) as ptp,
            tc.tile_pool(name="nrm", bufs=2) as nrm,
            tc.tile_pool(name="ypool", bufs=2) as yp,
        ):
            # ---------------- persistent tiles ----------------
            xT_sb = pp.tile([128, nct, n_tok], BF16, tag="xT")
            for ct in range(nct):
                nc.sync.dma_start(xT_sb[:, ct, :], xT_t[:, ct, :])
            qT_sb = pp.tile([128, npair, n_tok], BF16, tag="qT")
            kT_sb = pp.tile([128, npair, n_tok], BF16, tag="kT")
            v_sb = pp.tile([128, ntile, H, D + 1], BF16, tag="v")
            nc.gpsimd.memset(v_sb[:, :, :, D], 1.0)
            oT_sb = pp.tile([128, nct, n_tok], BF16, tag="oT")
            cos_sb = pp.tile([128, ntile, D], F32, tag="cos")
            nc.sync.dma_start(cos_sb[:], cos_t)
            sin_sb = pp.tile([128, ntile, D], F32, tag="sin")
            nc.sync.dma_start(sin_sb[:], sin_t)
            bias_b = pp.tile([128, C], F32, tag="biasb")
            pb_sb = pp.tile([1, C], F32, tag="pb")
            nc.sync.dma_start(pb_sb[:], pbias[:])
            nc.gpsimd.partition_broadcast(bias_b[:], pb_sb[0:1, :])
            pwc = pp.tile([128, nct, 2, 512], BF16, tag="pw")

            # -------------- 512-wide x-stationary chunk ---------------------
            def qkv_chunk(which, half):
                # which: 0=q 1=k 2=v; half: 0/1 -> features 512*half..+512
                # q/k chunks cover pairs 4*half..4*half+4 and get RoPE +
                # DMA-transpose into qT_sb/kT_sb; v chunks drain to v_sb.
                w = wsp.tile([128, nct, 512], BF16, tag="w")
                base = which * C + half * 512
                nc.sync.dma_start(w[:], wT_t[:, :, base : base + 512])
                for t in range(ntile):
                    pq = psA.tile([128, 512], F32, tag="pA")
                    for ct in range(nct):
                        nc.tensor.matmul(
                            pq[:],
                            xT_sb[:, ct, t * 128 : (t + 1) * 128],
                            w[:, ct, :],
                            start=(ct == 0),
                            stop=(ct == nct - 1),
                        )
                    if which == 2:
                        nc.vector.tensor_copy(
                            v_sb[:, t, half * 8 : half * 8 + 8, 0:D],
                            pq[:].rearrange("p (h d) -> p h d", d=D),
                        )
                        continue
                    pq3 = pq[:].rearrange("p (h d) -> p h d", d=D)
                    cos3 = (
                        cos_sb[:, t, :]
                        .rearrange("p (o d) -> p o d", d=D)
                        .to_broadcast([128, 8, D])
                    )
                    sinlo = (
                        sin_sb[:, t, 0:HD2]
                        .rearrange("p (o d) -> p o d", d=HD2)
                        .to_broadcast([128, 8, HD2])
                    )
                    sinhi = (
                        sin_sb[:, t, HD2:D]
                        .rearrange("p (o d) -> p o d", d=HD2)
                        .to_broadcast([128, 8, HD2])
                    )
                    tmp = rtp.tile([128, 512], F32, tag="rt")
                    tmp3 = tmp[:].rearrange("p (h d) -> p h d", d=D)
                    nc.vector.tensor_tensor(
                        out=tmp3[:, :, 0:HD2], in0=pq3[:, :, HD2:D],
                        in1=sinlo, op=OP.mult,
                    )
                    nc.vector.tensor_tensor(
                        out=tmp3[:, :, HD2:D], in0=pq3[:, :, 0:HD2],
                        in1=sinhi, op=OP.mult,
                    )
                    u = rtp.tile([128, 512], F32, tag="ru")
                    nc.vector.tensor_tensor(
                        out=u[:].rearrange("p (h d) -> p h d", d=D),
                        in0=pq3, in1=cos3, op=OP.mult,
                    )
                    qh = qsp.tile([128, 512], BF16, tag="qh")
                    nc.vector.tensor_tensor(
                        out=qh[:], in0=u[:], in1=tmp[:], op=OP.add
                    )
                    dstT = qT_sb if which == 0 else kT_sb
                    nc.sync.dma_start_transpose(
                        dstT[:, 4 * half : 4 * half + 4, t * 128 : (t + 1) * 128],
                        qh[:],
                    )

            # -------------- S^T + exp for one (pair, mc) --------------------
            def s_exp(p, mc):
                pT = ptp.tile([128, ntile, 2, 512], BF16, tag="pT")
                ms = mc * mch
                for t in range(ntile):
                    ps = psS.tile([128, 2, 512], F32, tag="ps")
                    nc.tensor.matmul(
                        ps[:, 0, :],
                        kT_sb[0:64, p, t * 128 : (t + 1) * 128],
                        qT_sb[0:64, p, ms : ms + mch],
                        start=True,
                        stop=True,
                    )
                    nc.tensor.matmul(
                        ps[:, 1, :],
                        kT_sb[64:128, p, t * 128 : (t + 1) * 128],
                        qT_sb[64:128, p, ms : ms + mch],
                        start=True,
                        stop=True,
                    )
                    nc.scalar.activation(
                        pT[:, t, :, :].rearrange("p a m -> p (a m)"),
                        ps[:].rearrange("p a m -> p (a m)"),
                        AF.Exp,
                        scale=SCALE,
                    )
                return pT

            # -------------- PV + normalize for one (pair, mc) ---------------
            def pv_norm(p, mc, pT):
                ms = mc * mch
                po = psPV.tile([128, 2, 512], F32, tag="po")
                for t in range(ntile):
                    nc.tensor.matmul(
                        po[0:65, 0, :],
                        v_sb[:, t, 2 * p, :],
                        pT[:, t, 0, :],
                        start=(t == 0),
                        stop=(t == ntile - 1),
                    )
                    nc.tensor.matmul(
                        po[0:65, 1, :],
                        v_sb[:, t, 2 * p + 1, :],
                        pT[:, t, 1, :],
                        start=(t == 0),
                        stop=(t == ntile - 1),
                    )
                with tc.high_priority():
                    ssb = nrm.tile([65, 2, 512], F32, tag="ssb")
                    nc.scalar.copy(ssb[64:65, :, :], po[64:65, :, :])
                    s0 = nrm.tile([1, 2, 512], F32, tag="s0")
                    nc.sync.dma_start(s0[:], ssb[64:65, :, :])
                    rs0 = nrm.tile([1, 2, 512], F32, tag="rs0")
                    nc.vector.reciprocal_approx_fast(out=rs0[:], in_=s0[:])
                    rbi = nrm.tile([64, 2, 512], F32, tag="rbi")
                    nc.gpsimd.partition_broadcast(rbi[:], rs0[0:1, :, :])
                    nc.vector.tensor_tensor(
                        out=oT_sb[0:64, p, ms : ms + mch],
                        in0=po[0:64, 0, :],
                        in1=rbi[:, 0, :],
                        op=OP.mult,
                    )
                    tmpo = nrm.tile([64, 512], BF16, tag="tmpo")
                    nc.vector.tensor_tensor(
                        out=tmpo[:],
                        in0=po[0:64, 1, :],
                        in1=rbi[:, 1, :],
                        op=OP.mult,
                    )
                    nc.sync.dma_start(oT_sb[64:128, p, ms : ms + mch], tmpo[:])

            # -------------- fused pipeline ----------------------------------
            qkv_chunk(0, 0)
            qkv_chunk(1, 0)
            qkv_chunk(2, 0)
            for p in range(npair):
                # Interleaved chunks for pairs 4-7 run at the LOWEST priority:
                # the scheduler uses their matmuls purely as PE-gap filler so
                # they never starve the ACT-paced softmax stream.
                if p == 0:
                    with tc.high_priority(offset=-(1 << 20)):
                        qkv_chunk(0, 1)
                if p == 1:
                    with tc.high_priority(offset=-(1 << 20)):
                        qkv_chunk(1, 1)
                if p == 2:
                    with tc.high_priority(offset=-(1 << 20)):
                        qkv_chunk(2, 1)
                if p == 3:
                    nc.sync.dma_start(
                        pwc[:], pwT_t.rearrange("p t (a e) -> p t a e", a=2)
                    )
                pT0 = s_exp(p, 0)
                pv_norm(p, 0, pT0)
                pT1 = s_exp(p, 1)
                pv_norm(p, 1, pT1)

            # -------------- proj tail ---------------------------------------
            for t in range(ntile):
                for ec in range(2):
                    py = psA.tile([128, 512], F32, tag="pA")
                    for ft in range(nct):
                        nc.tensor.matmul(
                            py[:],
                            oT_sb[:, ft, t * 128 : (t + 1) * 128],
                            pwc[:, ft, ec, :],
                            start=(ft == 0),
                            stop=(ft == nct - 1),
                        )
                    ysb = yp.tile([128, 512], F32, tag="y")
                    nc.vector.tensor_tensor(
                        out=ysb[:],
                        in0=py[:],
                        in1=bias_b[:, ec * 512 : (ec + 1) * 512],
                        op=OP.add,
                    )
                    nc.sync.dma_start(
                        y[t * 128 : (t + 1) * 128, ec * 512 : (ec + 1) * 512],
                        ysb[:],
                    )

    nc.compile()
    return nc


def _host_inputs(x, rope_freqs, qkv_w, proj_w, proj_b):
    import ml_dtypes

    x = np.asarray(x, dtype=np.float32)
    rope_freqs = np.asarray(rope_freqs, dtype=np.float32)
    qkv_w = np.asarray(qkv_w, dtype=np.float32)
    proj_w = np.asarray(proj_w, dtype=np.float32)
    proj_b = np.asarray(proj_b, dtype=np.float32)

    B, n_tok, _ = x.shape
    wTh = np.ascontiguousarray(qkv_w.T).astype(ml_dtypes.bfloat16)
    pwTh = np.ascontiguousarray(proj_w.T).astype(ml_dtypes.bfloat16)
    freqs = rope_freqs[0, :, 0, :]  # [N, D]
    cosh = np.cos(freqs).astype(np.float32)
    sinh = np.sin(freqs).astype(np.float32)
    sinAh = np.concatenate([-sinh[:, :HD2], sinh[:, HD2:]], axis=1)
    sinAh = np.ascontiguousarray(sinAh)
    pbh = np.ascontiguousarray(proj_b.reshape(1, C))

    in_maps = []
    for b in range(B):
        in_maps.append(
            {
                "xT": np.ascontiguousarray(x[b].T).astype(ml_dtypes.bfloat16),
                "wT": wTh,
                "pwT": pwTh,
                "pbias": pbh,
                "cosN": cosh,
                "sinA": sinAh,
            }
        )
    return in_maps, n_tok


def kernel(x, rope_freqs, qkv_w, proj_w, proj_b):
    global LAST_EXEC_NS
    in_maps, n_tok = _host_inputs(x, rope_freqs, qkv_w, proj_w, proj_b)
    key = ("nc", n_tok)
    if key not in _CACHE:
        _CACHE[key] = build(n_tok)
    nc = _CACHE[key]

    trace = False
    if PROFILE:
        try:
            import profshim

            profshim.install()
            trace = True
        except Exception:
            trace = False

    res = run_bass_kernel_spmd(
        nc, in_maps, list(range(len(in_maps))), trace=trace
    )
    LAST_EXEC_NS = res.exec_time_ns
    out = np.stack([res.results[b]["y"] for b in range(len(in_maps))], axis=0)
    return out
